# revision 12
# baseline (speedup 1.0000x reference)
"""Trainium2 Bass kernel for nn_Attention_21878563405851.

Module: kv = x1 @ W_qk (k,v split); q = x2 @ W_v; 8-head attention
(dim_head=64); out @ W_out + b_out.  B=2, N=2048, DIM=512.

Sharding over 8 NeuronCores: core c -> batch b=c//4, head pair
g=c%4 (heads 2g, 2g+1), ALL 2048 queries.  Tensor-parallel over
heads: every projection (q, k, v, out) is computed exactly once
system-wide -- no replication.  W_out is row-sharded; each core
emits a partial y^T and the 4-way reduction per batch happens on
the host during unshard (cheaper than this fabric's collectives).

Per core:
  1. x1/x2 loaded as single full-tensor DMAs (16KB contiguous run
     per partition -- large packets; the 4KB-run chunked form ran
     at ~60-80 GB/s/queue and dominated the old 27us lead-in).
  2. k proj half0 + q proj qc0, then 8 attention blocks (qc, h):
     dots^T[kt] = k_h @ q_h^T -> exp (ACT, [128,1024] pair tiles,
     scale folded) -> attnv into [65, 512] PSUM (row 64 =
     denominator via a ones column in v).  All other PE work
     (k half1, q qc1-3, key-major v proj, attnv pairs, out proj)
     is drained from a global FIFO behind the dots/exp stream so
     the in-order PE queue always has ready work while ACT runs
     the ~1.1us exps.
  3. normalization: denominator row -> SBUF, reciprocal_approx_fast
     (cannot read PSUM on hw), bf16, then partition-broadcast via a
     PE rank-1 matmul (ones[1,64] x r[1,512] -> PSUM, ~0.2us vs
     1.9us on GpSimd -- this chain is tail-critical), DVE multiply
     into o^T (bf16).
  4. out proj per qc: yp [128 d, 512 q] PSUM -> y_sb bf16; output
     DMA'd per qc (contiguous 4KB runs), last qc split per-dg so
     the final piece is small.

PSUM (8 banks): "big" [128,1024]x2 for k-proj halves + dots pairs;
"s5" [128,512]x2 for q/v proj, broadcast tiles and out-proj
partials; "acc" [128,512]x2 for the long-lived attnv accumulators.
"""

import sys
from collections import deque

for _p in ("/opt/trn_rl_repo", "/root/.axon_site/_ro/trn_rl_repo"):
    if _p not in sys.path:
        sys.path.insert(0, _p)

import numpy as np
import ml_dtypes

import concourse.mybir as mybir
from concourse import tile
from concourse.bacc import Bacc

B, N, DIM = 2, 2048, 512
HEADS, DH = 8, 64
INNER = HEADS * DH
SCALE = DH ** -0.5
NCORES = 8
NKT = N // 128     # 16 key tiles
NC = DIM // 128    # 4 contraction chunks

BF16 = mybir.dt.bfloat16
F32 = mybir.dt.float32


def build_program():
    nc = Bacc(None, num_devices=NCORES)

    # ---- external I/O (per core), host-prearranged SBUF images ----
    x1T = nc.dram_tensor("x1T", [128, NC * N], BF16, kind="ExternalInput")
    x2T = nc.dram_tensor("x2T", [128, NC * N], BF16, kind="ExternalInput")
    wk = nc.dram_tensor("wk", [128, NC * 128], BF16, kind="ExternalInput")
    wq = nc.dram_tensor("wq", [128, NC * 128], BF16, kind="ExternalInput")
    wv = nc.dram_tensor("wv", [128, NC * 128], BF16, kind="ExternalInput")
    wo = nc.dram_tensor("wo", [128, 4 * 128], BF16, kind="ExternalInput")
    # [p, (qc dg q)] bf16 partial output
    yT = nc.dram_tensor("yT", [128, 4 * N], BF16, kind="ExternalOutput")

    with tile.TileContext(nc) as tc:
        with (
            tc.tile_pool(name="xin", bufs=1) as xin,
            tc.tile_pool(name="wts", bufs=1) as wts,
            tc.tile_pool(name="kq", bufs=1) as kqp,
            tc.tile_pool(name="vex", bufs=1) as vexp,
            tc.tile_pool(name="et", bufs=20) as etp,
            tc.tile_pool(name="os", bufs=1) as osp,
            tc.tile_pool(name="ysb", bufs=1) as ysbp,
            tc.tile_pool(name="nrm", bufs=2) as nrmp,
            tc.tile_pool(name="ps", bufs=1, space="PSUM") as psp,
        ):
            # ---- load inputs: one big transfer per HW queue ----
            x1T_s = xin.tile([128, NC * N], BF16, name="x1T_s")
            x2T_s = xin.tile([128, NC * N], BF16, name="x2T_s")
            wq_s = wts.tile([128, NC * 128], BF16, name="wq_s")
            wk_s = wts.tile([128, NC * 128], BF16, name="wk_s")
            wv_s = wts.tile([128, NC * 128], BF16, name="wv_s")
            wo_s = wts.tile([128, 4 * 128], BF16, name="wo_s")

            nc.gpsimd.dma_start(wk_s[:], wk[:])
            nc.gpsimd.dma_start(wq_s[:], wq[:])
            nc.sync.dma_start(x1T_s[:], x1T[:])
            nc.scalar.dma_start(x2T_s[:], x2T[:])
            nc.gpsimd.dma_start(wv_s[:], wv[:])
            nc.gpsimd.dma_start(wo_s[:], wo[:])

            qT_s = kqp.tile([128, N], BF16, name="qT_s")
            kT_s = kqp.tile([128, N], BF16, name="kT_s")
            # v extended: per key tile, per head: 64 v cols + 1 ones col
            vE_s = vexp.tile([128, NKT, 2, 65], BF16, name="vE_s")
            nc.vector.memset(vE_s[:, :, :, 64:65], 1.0)
            ones_s = vexp.tile([1, 64], BF16, name="ones_s")
            nc.vector.memset(ones_s[:], 1.0)

            o_s = osp.tile([128, N], BF16, name="o_s")
            y_sb = ysbp.tile([128, 4, 4, 512], BF16, name="y_sb")

            def q_proj(t):
                qp = psp.tile([128, 512], F32, name=f"qp{t}", tag="s5", bufs=2)
                for c in range(NC):
                    nc.tensor.matmul(
                        qp[:],
                        wq_s[:, c * 128:(c + 1) * 128],
                        x2T_s[:, c * N + t * 512: c * N + (t + 1) * 512],
                        start=(c == 0),
                        stop=(c == NC - 1),
                    )
                nc.vector.tensor_copy(qT_s[:, t * 512:(t + 1) * 512], qp[:])

            def k_half(half):
                kh = psp.tile([128, 1024], F32, name=f"kh{half}", tag="big",
                              bufs=2)
                for c in range(NC):
                    for j in range(2):
                        col = half * 1024 + j * 512
                        nc.tensor.matmul(
                            kh[:, j * 512:(j + 1) * 512],
                            wk_s[:, c * 128:(c + 1) * 128],
                            x1T_s[:, c * N + col: c * N + col + 512],
                            start=(c == 0),
                            stop=(c == NC - 1),
                        )
                # split drain so the first dots only waits on 512 cols
                for j in range(2):
                    nc.vector.tensor_copy(
                        kT_s[:, half * 1024 + j * 512:
                             half * 1024 + (j + 1) * 512],
                        kh[:, j * 512:(j + 1) * 512],
                    )

            def v_pair(p):
                # key-major v for kt = 2p, 2p+1 (both heads + ones col)
                for j in range(2):
                    kt = 2 * p + j
                    vp = psp.tile([128, 128], F32, name="vp", tag="s5", bufs=2)
                    for c in range(NC):
                        nc.tensor.matmul(
                            vp[:],
                            x1T_s[:, c * N + kt * 128: c * N + (kt + 1) * 128],
                            wv_s[:, c * 128:(c + 1) * 128],
                            start=(c == 0),
                            stop=(c == NC - 1),
                        )
                    nc.vector.tensor_copy(
                        vE_s[:, kt, :, 0:64],
                        vp.rearrange("p (h d) -> p h d", h=2),
                    )

            # ---- PE pstate warm-up: dummy matmuls on the weights while
            # x1/x2 stream in, so the real projections run at full clock
            # (first matmuls after idle run at the 1.2GHz mid pstate) ----
            warm = psp.tile([128, 512], F32, name="warm", tag="s5", bufs=2)
            for _ in range(16):
                nc.tensor.matmul(warm[:], wk_s[:, 0:128], wq_s[:])

            k_half(0)
            q_proj(0)

            # ---- global deferred-PE-work FIFO ----
            fifo = deque()
            emitted = set()

            def run(e):
                e[2]()
                emitted.add(e[3])

            fifo.append((0, 2, lambda: k_half(1), "kh1"))
            fifo.append((0, 2, lambda: v_pair(0), "v0"))
            fifo.append((0, 2, lambda: v_pair(1), "v1"))
            for t in (1, 2, 3):
                fifo.append((0, 1, lambda t=t: q_proj(t), f"q{t}"))
            for p in (2, 3, 4, 5, 6, 7):
                fifo.append((0, 2, lambda p=p: v_pair(p), f"v{p}"))

            # ---- attention: 8 blocks of (qc, h) ----
            ET_BUFS = 20

            for blk in range(8):
                qc, h = blk // 2, blk % 2
                r0 = h * 64
                acc = psp.tile([128, 512], F32, name=f"acc{blk}", tag="acc",
                               bufs=2)

                def emit_attnv(kp, e_t, acc=acc, h=h):
                    for j in range(2):
                        kt = 2 * kp + j
                        nc.tensor.matmul(
                            acc[0:65, :],
                            vE_s[:, kt, h, :],
                            e_t[:, j * 512:(j + 1) * 512],
                            start=(kt == 0),
                            stop=(kt == NKT - 1),
                        )

                for kp in range(8):
                    step = blk * 8 + kp
                    dt = psp.tile([128, 1024], F32, name="dt", tag="big",
                                  bufs=2)
                    for j in range(2):
                        kt = 2 * kp + j
                        nc.tensor.matmul(
                            dt[:, j * 512:(j + 1) * 512],
                            kT_s[r0:r0 + 64, kt * 128:(kt + 1) * 128],
                            qT_s[r0:r0 + 64, qc * 512:(qc + 1) * 512],
                        )
                    e_t = etp.tile([128, 1024], BF16, name="e_t", tag="e")
                    nc.scalar.activation(
                        e_t[:], dt[:],
                        mybir.ActivationFunctionType.Exp, scale=SCALE,
                    )
                    # block0's attnv is deferred further so the early steps
                    # have room for the v-proj fillers without starving ACT
                    fifo.append((step + (6 if blk == 0 else 2), 1,
                                 lambda kp=kp, e_t=e_t, f=emit_attnv:
                                 f(kp, e_t),
                                 f"av{blk}_{kp}"))

                    # mandatory pops: e_t ring safety + norm-lag bound
                    while fifo and (
                        (step >= ET_BUFS - 2 and
                         f"av{(step - ET_BUFS + 2) // 8}_"
                         f"{(step - ET_BUFS + 2) % 8}" not in emitted)
                        or (kp == 0 and blk >= 2 and
                            f"mult{blk - 2}" not in emitted)
                    ):
                        run(fifo.popleft())
                    # budgeted pops (~1 matmul-pair of PE work per step)
                    budget = 2
                    while fifo and budget > 0 and fifo[0][0] <= step:
                        e = fifo.popleft()
                        budget -= e[1]
                        run(e)

                # normalization chain; broadcast via PE rank-1 matmul
                rb_box = []

                def norm_run(acc=acc, rb_box=rb_box):
                    s_s = nrmp.tile([1, 512], F32, name="s_s", tag="s")
                    nc.vector.tensor_copy(s_s[:], acc[64:65, :])
                    r_s = nrmp.tile([1, 512], F32, name="r_s", tag="r")
                    nc.vector.reciprocal_approx_fast(r_s[:], s_s[:])
                    r16 = nrmp.tile([1, 512], BF16, name="r16", tag="r16")
                    nc.vector.tensor_copy(r16[:], r_s[:])
                    rb_ps = psp.tile([64, 512], F32, name="rb_ps", tag="s5",
                                     bufs=2)
                    nc.tensor.matmul(rb_ps[:], ones_s[:], r16[:])
                    # DVE can't read two PSUM operands in one tensor_tensor
                    rb_s = nrmp.tile([64, 512], BF16, name="rb_s", tag="rb")
                    nc.vector.tensor_copy(rb_s[:], rb_ps[:])
                    rb_box.append(rb_s)

                def emit_mult(qc=qc, r0=r0, acc=acc, rb_box=rb_box):
                    nc.vector.tensor_mul(
                        o_s[r0:r0 + 64, qc * 512:(qc + 1) * 512],
                        acc[0:64, :], rb_box[0][:],
                    )

                last = blk * 8 + 7
                fifo.append((last + 2, 0, norm_run, f"norm{blk}"))
                fifo.append((last + 3, 0, emit_mult, f"mult{blk}"))

                if h == 1:
                    def emit_y(qc=qc):
                        for dg in range(4):
                            yp = psp.tile([128, 512], F32, name=f"yp{qc}{dg}",
                                          tag="s5", bufs=2)
                            nc.tensor.matmul(
                                yp[:],
                                wo_s[:, dg * 128:(dg + 1) * 128],
                                o_s[:, qc * 512:(qc + 1) * 512],
                            )
                            nc.vector.tensor_copy(y_sb[:, qc, dg, :], yp[:])
                            if qc == 3:
                                # tail-critical: ship each dg as it drains
                                eng = (nc.sync, nc.gpsimd)[dg % 2]
                                eng.dma_start(
                                    yT[:, (qc * 4 + dg) * 512:
                                       (qc * 4 + dg + 1) * 512],
                                    y_sb[:, qc, dg, :],
                                )
                        if qc < 3:
                            # one contiguous [128, 4KB-run] transfer
                            eng = (nc.gpsimd, nc.sync, nc.gpsimd)[qc]
                            eng.dma_start(
                                yT[:, qc * 4 * 512:(qc + 1) * 4 * 512],
                                y_sb[:, qc, :, :],
                            )
                    fifo.append((last + 3, 2, emit_y, f"y{qc}"))

            # flush remaining deferred work
            while fifo:
                run(fifo.popleft())

    nc.finalize()
    return nc


_NC_CACHE = None


def _get_program():
    global _NC_CACHE
    if _NC_CACHE is None:
        _NC_CACHE = build_program()
    return _NC_CACHE


def make_in_maps(x1, x2, W_qk, W_v, W_out, b_out):
    bf = ml_dtypes.bfloat16
    x1 = np.asarray(x1, np.float32)
    x2 = np.asarray(x2, np.float32)
    W_qk = np.asarray(W_qk, np.float32)
    W_v = np.asarray(W_v, np.float32)
    W_out = np.asarray(W_out, np.float32)

    # [p, (c k)] images: X[b]^T with the 512-dim contraction split into
    # 4 chunks of 128 partitions
    def xT_img(X):
        return np.ascontiguousarray(
            X.reshape(N, NC, 128).transpose(2, 1, 0).reshape(128, NC * N)
        ).astype(bf)

    x1T_imgs = [xT_img(x1[b]) for b in range(B)]
    x2T_imgs = [xT_img(x2[b]) for b in range(B)]

    # weight images per head-pair g: [p, (c f)] = W[c*128+p, g*128+f]
    def w_img(W, g):
        return np.ascontiguousarray(
            W[:, g * 128:(g + 1) * 128]
            .reshape(NC, 128, 128).transpose(1, 0, 2).reshape(128, NC * 128)
        ).astype(bf)

    wk_imgs = [w_img(W_qk[:, :INNER], g) for g in range(4)]
    wv_imgs = [w_img(W_qk[:, INNER:], g) for g in range(4)]
    wq_imgs = [w_img(W_v, g) for g in range(4)]
    # wo: rows for this head pair, [p, (dg f)] = W_out[g*128+p, dg*128+f]
    wo_imgs = [
        np.ascontiguousarray(W_out[g * 128:(g + 1) * 128, :]).astype(bf)
        for g in range(4)
    ]

    in_maps = []
    for c in range(NCORES):
        b, g = c // 4, c % 4
        in_maps.append(
            {
                "x1T": x1T_imgs[b],
                "x2T": x2T_imgs[b],
                "wk": wk_imgs[g],
                "wq": wq_imgs[g],
                "wv": wv_imgs[g],
                "wo": wo_imgs[g],
            }
        )
    return in_maps


def assemble_output(results, b_out):
    y = np.zeros((B, N, DIM), np.float32)
    for c in range(NCORES):
        b = c // 4
        yTc = np.asarray(results[c]["yT"], np.float32)  # [128, (qc dg q)]
        # yTc[p, qc, dg, q] = y_part[qc*512+q, dg*128+p]
        D = yTc.reshape(128, 4, 4, 512)
        # -> [qc, q, dg, p] -> [N, DIM]
        y[b] += D.transpose(1, 3, 2, 0).reshape(N, DIM)
    y += np.asarray(b_out, np.float32)
    return y


def kernel(x1, x2, W_qk, W_v, W_out, b_out):
    from concourse.bass_utils import run_bass_kernel_spmd

    nc = _get_program()
    in_maps = make_in_maps(x1, x2, W_qk, W_v, W_out, b_out)
    res = run_bass_kernel_spmd(nc, in_maps, list(range(NCORES)))
    return assemble_output(res.results, b_out)


# revision 18
# speedup vs baseline: 1.0126x; 1.0126x over previous
"""Trainium2 Bass kernel for nn_Attention_21878563405851.

Module: kv = x1 @ W_qk (k,v split); q = x2 @ W_v; 8-head attention
(dim_head=64); out @ W_out + b_out.  B=2, N=2048, DIM=512.

Sharding over 8 NeuronCores: core c -> batch b=c//4, head pair
g=c%4 (heads 2g, 2g+1), ALL 2048 queries.  Tensor-parallel over
heads: every projection (q, k, v, out) is computed exactly once
system-wide -- no replication.  W_out is row-sharded; each core
emits a partial y^T and the 4-way reduction per batch happens on
the host during unshard (cheaper than this fabric's collectives).

Per core:
  1. x1/x2 loaded as single full-tensor DMAs (16KB contiguous run
     per partition -- large packets; the 4KB-run chunked form ran
     at ~60-80 GB/s/queue and dominated the old 27us lead-in).
  2. k proj half0 + q proj qc0, then 8 attention blocks (qc, h):
     dots^T[kt] = k_h @ q_h^T -> exp (ACT, [128,1024] pair tiles,
     scale folded) -> attnv into [65, 512] PSUM (row 64 =
     denominator via a ones column in v).  All other PE work
     (k half1, q qc1-3, key-major v proj, attnv pairs, out proj)
     is drained from a global FIFO behind the dots/exp stream so
     the in-order PE queue always has ready work while ACT runs
     the ~1.1us exps.
  3. normalization: denominator row -> SBUF, reciprocal_approx_fast
     (cannot read PSUM on hw), bf16, then partition-broadcast via a
     PE rank-1 matmul (ones[1,64] x r[1,512] -> PSUM, ~0.2us vs
     1.9us on GpSimd -- this chain is tail-critical), DVE multiply
     into o^T (bf16).
  4. out proj per qc: yp [128 d, 512 q] PSUM -> y_sb bf16; output
     DMA'd per qc (contiguous 4KB runs), last qc split per-dg so
     the final piece is small.

PSUM (8 banks): "big" [128,1024]x2 for k-proj halves + dots pairs;
"s5" [128,512]x2 for q/v proj, broadcast tiles and out-proj
partials; "acc" [128,512]x2 for the long-lived attnv accumulators.
"""

import sys
from collections import deque

for _p in ("/opt/trn_rl_repo", "/root/.axon_site/_ro/trn_rl_repo"):
    if _p not in sys.path:
        sys.path.insert(0, _p)

import numpy as np
import ml_dtypes

import concourse.mybir as mybir
from concourse import tile
from concourse.bacc import Bacc

B, N, DIM = 2, 2048, 512
HEADS, DH = 8, 64
INNER = HEADS * DH
SCALE = DH ** -0.5
NCORES = 8
NKT = N // 128     # 16 key tiles
NC = DIM // 128    # 4 contraction chunks

BF16 = mybir.dt.bfloat16
F32 = mybir.dt.float32


def build_program():
    nc = Bacc(None, num_devices=NCORES)

    # ---- external I/O (per core), host-prearranged SBUF images ----
    x1T = nc.dram_tensor("x1T", [128, NC * N], BF16, kind="ExternalInput")
    x2T = nc.dram_tensor("x2T", [128, NC * N], BF16, kind="ExternalInput")
    wk = nc.dram_tensor("wk", [128, NC * 128], BF16, kind="ExternalInput")
    wq = nc.dram_tensor("wq", [128, NC * 128], BF16, kind="ExternalInput")
    wv = nc.dram_tensor("wv", [128, NC * 128], BF16, kind="ExternalInput")
    wo = nc.dram_tensor("wo", [128, 4 * 128], BF16, kind="ExternalInput")
    # [p, (qc dg q)] bf16 partial output
    yT = nc.dram_tensor("yT", [128, 4 * N], BF16, kind="ExternalOutput")

    with tile.TileContext(nc) as tc:
        with (
            tc.tile_pool(name="xin", bufs=1) as xin,
            tc.tile_pool(name="wts", bufs=1) as wts,
            tc.tile_pool(name="kq", bufs=1) as kqp,
            tc.tile_pool(name="vex", bufs=1) as vexp,
            tc.tile_pool(name="et", bufs=20) as etp,
            tc.tile_pool(name="os", bufs=1) as osp,
            tc.tile_pool(name="ysb", bufs=1) as ysbp,
            tc.tile_pool(name="nrm", bufs=2) as nrmp,
            tc.tile_pool(name="ps", bufs=1, space="PSUM") as psp,
        ):
            # ---- load inputs: one big transfer per HW queue ----
            x1T_s = xin.tile([128, NC * N], BF16, name="x1T_s")
            x2T_s = xin.tile([128, NC * N], BF16, name="x2T_s")
            wq_s = wts.tile([128, NC * 128], BF16, name="wq_s")
            wk_s = wts.tile([128, NC * 128], BF16, name="wk_s")
            wv_s = wts.tile([128, NC * 128], BF16, name="wv_s")
            wo_s = wts.tile([128, 4 * 128], BF16, name="wo_s")

            # wk/wq ride the fast HW queues ahead of the bulk tensors
            # (the SW gpsimd queue only delivers ~0.5MB by ~20us, too late
            # for the k/q projections); wv/wo are needed later and stay SW.
            nc.sync.dma_start(wk_s[:], wk[:])
            nc.scalar.dma_start(wq_s[:], wq[:])
            nc.sync.dma_start(x1T_s[:], x1T[:])
            nc.scalar.dma_start(x2T_s[:], x2T[:])
            nc.gpsimd.dma_start(wv_s[:], wv[:])
            nc.gpsimd.dma_start(wo_s[:], wo[:])

            qT_s = kqp.tile([128, N], BF16, name="qT_s")
            kT_s = kqp.tile([128, N], BF16, name="kT_s")
            # v extended: per key tile, per head: 64 v cols + 1 ones col
            vE_s = vexp.tile([128, NKT, 2, 65], BF16, name="vE_s")
            nc.vector.memset(vE_s[:, :, :, 64:65], 1.0)
            ones_s = vexp.tile([1, 64], BF16, name="ones_s")
            nc.vector.memset(ones_s[:], 1.0)

            o_s = osp.tile([128, N], BF16, name="o_s")
            y_sb = ysbp.tile([128, 4, 4, 512], BF16, name="y_sb")

            def q_proj(t):
                qp = psp.tile([128, 512], F32, name=f"qp{t}", tag="s5", bufs=2)
                for c in range(NC):
                    nc.tensor.matmul(
                        qp[:],
                        wq_s[:, c * 128:(c + 1) * 128],
                        x2T_s[:, c * N + t * 512: c * N + (t + 1) * 512],
                        start=(c == 0),
                        stop=(c == NC - 1),
                    )
                nc.vector.tensor_copy(qT_s[:, t * 512:(t + 1) * 512], qp[:])

            kh_box = {}

            def k_part(half, cs):
                # one accumulation group split into two emission parts so
                # the FIFO can interleave them; cs = (0, 1) or (2, 3)
                if half not in kh_box:
                    kh_box[half] = psp.tile([128, 1024], F32,
                                            name=f"kh{half}", tag="big",
                                            bufs=2)
                kh = kh_box[half]
                for c in cs:
                    for j in range(2):
                        col = half * 1024 + j * 512
                        nc.tensor.matmul(
                            kh[:, j * 512:(j + 1) * 512],
                            wk_s[:, c * 128:(c + 1) * 128],
                            x1T_s[:, c * N + col: c * N + col + 512],
                            start=(c == 0),
                            stop=(c == NC - 1),
                        )
                if cs[-1] == NC - 1:
                    # split drain so the first dots only waits on 512 cols
                    for j in range(2):
                        nc.vector.tensor_copy(
                            kT_s[:, half * 1024 + j * 512:
                                 half * 1024 + (j + 1) * 512],
                            kh[:, j * 512:(j + 1) * 512],
                        )

            def k_half(half):
                k_part(half, (0, 1))
                k_part(half, (2, 3))

            def v_pair(p):
                # key-major v for kt = 2p, 2p+1 (both heads + ones col)
                for j in range(2):
                    kt = 2 * p + j
                    vp = psp.tile([128, 128], F32, name="vp", tag="s5", bufs=2)
                    for c in range(NC):
                        nc.tensor.matmul(
                            vp[:],
                            x1T_s[:, c * N + kt * 128: c * N + (kt + 1) * 128],
                            wv_s[:, c * 128:(c + 1) * 128],
                            start=(c == 0),
                            stop=(c == NC - 1),
                        )
                    nc.vector.tensor_copy(
                        vE_s[:, kt, :, 0:64],
                        vp.rearrange("p (h d) -> p h d", h=2),
                    )

            # ---- PE pstate warm-up: dummy matmuls on the weights while
            # x1/x2 stream in, so the real projections run at full clock
            # (first matmuls after idle run at the 1.2GHz mid pstate) ----
            warm = psp.tile([128, 512], F32, name="warm", tag="s5", bufs=2)
            for _ in range(28):
                nc.tensor.matmul(warm[:], wk_s[:, 0:128], wk_s[:])

            k_half(0)
            q_proj(0)

            # ---- global deferred-PE-work FIFO ----
            fifo = deque()
            emitted = set()

            def run(e):
                e[2]()
                emitted.add(e[3])

            fifo.append((0, 1, lambda: k_part(1, (0, 1)), "kh1a"))
            fifo.append((0, 1, lambda: k_part(1, (2, 3)), "kh1b"))
            fifo.append((0, 2, lambda: v_pair(0), "v0"))
            fifo.append((0, 2, lambda: v_pair(1), "v1"))
            for t in (1, 2, 3):
                fifo.append((0, 1, lambda t=t: q_proj(t), f"q{t}"))
            for p in (2, 3, 4, 5, 6, 7):
                fifo.append((0, 2, lambda p=p: v_pair(p), f"v{p}"))

            # ---- attention: 8 blocks of (qc, h) ----
            ET_BUFS = 20

            for blk in range(8):
                qc, h = blk // 2, blk % 2
                r0 = h * 64
                acc = psp.tile([128, 512], F32, name=f"acc{blk}", tag="acc",
                               bufs=2)

                def emit_attnv(kp, e_t, acc=acc, h=h):
                    for j in range(2):
                        kt = 2 * kp + j
                        nc.tensor.matmul(
                            acc[0:65, :],
                            vE_s[:, kt, h, :],
                            e_t[:, j * 512:(j + 1) * 512],
                            start=(kt == 0),
                            stop=(kt == NKT - 1),
                        )

                for kp in range(8):
                    step = blk * 8 + kp
                    dt = psp.tile([128, 1024], F32, name="dt", tag="big",
                                  bufs=2)
                    for j in range(2):
                        kt = 2 * kp + j
                        nc.tensor.matmul(
                            dt[:, j * 512:(j + 1) * 512],
                            kT_s[r0:r0 + 64, kt * 128:(kt + 1) * 128],
                            qT_s[r0:r0 + 64, qc * 512:(qc + 1) * 512],
                        )
                    e_t = etp.tile([128, 1024], BF16, name="e_t", tag="e")
                    nc.scalar.activation(
                        e_t[:], dt[:],
                        mybir.ActivationFunctionType.Exp, scale=SCALE,
                    )
                    # block0's attnv is deferred further so the early steps
                    # have room for the v-proj fillers without starving ACT
                    fifo.append((step + (6 if blk == 0 else 2), 1,
                                 lambda kp=kp, e_t=e_t, f=emit_attnv:
                                 f(kp, e_t),
                                 f"av{blk}_{kp}"))

                    # mandatory pops: e_t ring safety + norm-lag bound
                    while fifo and (
                        (step >= ET_BUFS - 2 and
                         f"av{(step - ET_BUFS + 2) // 8}_"
                         f"{(step - ET_BUFS + 2) % 8}" not in emitted)
                        or (kp == 0 and blk >= 2 and
                            f"mult{blk - 2}" not in emitted)
                    ):
                        run(fifo.popleft())
                    # budgeted pops (~1 matmul-pair of PE work per step)
                    budget = 2
                    while fifo and budget > 0 and fifo[0][0] <= step:
                        e = fifo.popleft()
                        budget -= e[1]
                        run(e)

                # normalization chain; broadcast via PE rank-1 matmul
                rb_box = []

                def norm_run(acc=acc, rb_box=rb_box):
                    s_s = nrmp.tile([1, 512], F32, name="s_s", tag="s")
                    nc.vector.tensor_copy(s_s[:], acc[64:65, :])
                    r_s = nrmp.tile([1, 512], F32, name="r_s", tag="r")
                    nc.vector.reciprocal_approx_fast(r_s[:], s_s[:])
                    r16 = nrmp.tile([1, 512], BF16, name="r16", tag="r16")
                    nc.vector.tensor_copy(r16[:], r_s[:])
                    rb_ps = psp.tile([64, 512], F32, name="rb_ps", tag="s5",
                                     bufs=2)
                    nc.tensor.matmul(rb_ps[:], ones_s[:], r16[:])
                    # DVE can't read two PSUM operands in one tensor_tensor
                    rb_s = nrmp.tile([64, 512], BF16, name="rb_s", tag="rb")
                    nc.vector.tensor_copy(rb_s[:], rb_ps[:])
                    rb_box.append(rb_s)

                def emit_mult(qc=qc, r0=r0, acc=acc, rb_box=rb_box):
                    nc.vector.tensor_mul(
                        o_s[r0:r0 + 64, qc * 512:(qc + 1) * 512],
                        acc[0:64, :], rb_box[0][:],
                    )

                last = blk * 8 + 7
                fifo.append((last + 2, 0, norm_run, f"norm{blk}"))
                fifo.append((last + 3, 0, emit_mult, f"mult{blk}"))

                if h == 1:
                    def emit_y_dg(qc, dg):
                        yp = psp.tile([128, 512], F32, name=f"yp{qc}{dg}",
                                      tag="s5", bufs=2)
                        nc.tensor.matmul(
                            yp[:],
                            wo_s[:, dg * 128:(dg + 1) * 128],
                            o_s[:, qc * 512:(qc + 1) * 512],
                        )
                        nc.vector.tensor_copy(y_sb[:, qc, dg, :], yp[:])
                        if qc == 3:
                            # tail-critical: ship each dg as it drains
                            eng = (nc.sync, nc.gpsimd)[dg % 2]
                            eng.dma_start(
                                yT[:, (qc * 4 + dg) * 512:
                                   (qc * 4 + dg + 1) * 512],
                                y_sb[:, qc, dg, :],
                            )
                        elif dg == 3:
                            # one contiguous [128, 4KB-run] transfer
                            eng = (nc.gpsimd, nc.sync, nc.gpsimd)[qc]
                            eng.dma_start(
                                yT[:, qc * 4 * 512:(qc + 1) * 4 * 512],
                                y_sb[:, qc, :, :],
                            )
                    for dg in range(4):
                        fifo.append((last + 3 + dg, 1,
                                     lambda qc=qc, dg=dg: emit_y_dg(qc, dg),
                                     f"y{qc}_{dg}"))

            # flush remaining deferred work
            while fifo:
                run(fifo.popleft())

    nc.finalize()
    return nc


_NC_CACHE = None


def _get_program():
    global _NC_CACHE
    if _NC_CACHE is None:
        _NC_CACHE = build_program()
    return _NC_CACHE


def make_in_maps(x1, x2, W_qk, W_v, W_out, b_out):
    bf = ml_dtypes.bfloat16
    x1 = np.asarray(x1, np.float32)
    x2 = np.asarray(x2, np.float32)
    W_qk = np.asarray(W_qk, np.float32)
    W_v = np.asarray(W_v, np.float32)
    W_out = np.asarray(W_out, np.float32)

    # [p, (c k)] images: X[b]^T with the 512-dim contraction split into
    # 4 chunks of 128 partitions
    def xT_img(X):
        return np.ascontiguousarray(
            X.reshape(N, NC, 128).transpose(2, 1, 0).reshape(128, NC * N)
        ).astype(bf)

    x1T_imgs = [xT_img(x1[b]) for b in range(B)]
    x2T_imgs = [xT_img(x2[b]) for b in range(B)]

    # weight images per head-pair g: [p, (c f)] = W[c*128+p, g*128+f]
    def w_img(W, g):
        return np.ascontiguousarray(
            W[:, g * 128:(g + 1) * 128]
            .reshape(NC, 128, 128).transpose(1, 0, 2).reshape(128, NC * 128)
        ).astype(bf)

    wk_imgs = [w_img(W_qk[:, :INNER], g) for g in range(4)]
    wv_imgs = [w_img(W_qk[:, INNER:], g) for g in range(4)]
    wq_imgs = [w_img(W_v, g) for g in range(4)]
    # wo: rows for this head pair, [p, (dg f)] = W_out[g*128+p, dg*128+f]
    wo_imgs = [
        np.ascontiguousarray(W_out[g * 128:(g + 1) * 128, :]).astype(bf)
        for g in range(4)
    ]

    in_maps = []
    for c in range(NCORES):
        b, g = c // 4, c % 4
        in_maps.append(
            {
                "x1T": x1T_imgs[b],
                "x2T": x2T_imgs[b],
                "wk": wk_imgs[g],
                "wq": wq_imgs[g],
                "wv": wv_imgs[g],
                "wo": wo_imgs[g],
            }
        )
    return in_maps


def assemble_output(results, b_out):
    y = np.zeros((B, N, DIM), np.float32)
    for c in range(NCORES):
        b = c // 4
        yTc = np.asarray(results[c]["yT"], np.float32)  # [128, (qc dg q)]
        # yTc[p, qc, dg, q] = y_part[qc*512+q, dg*128+p]
        D = yTc.reshape(128, 4, 4, 512)
        # -> [qc, q, dg, p] -> [N, DIM]
        y[b] += D.transpose(1, 3, 2, 0).reshape(N, DIM)
    y += np.asarray(b_out, np.float32)
    return y


def kernel(x1, x2, W_qk, W_v, W_out, b_out):
    from concourse.bass_utils import run_bass_kernel_spmd

    nc = _get_program()
    in_maps = make_in_maps(x1, x2, W_qk, W_v, W_out, b_out)
    res = run_bass_kernel_spmd(nc, in_maps, list(range(NCORES)))
    return assemble_output(res.results, b_out)
